# revision 4
# baseline (speedup 1.0000x reference)
"""NT-Xent loss on 8 Trainium2 cores.

Math: with row-normalized views zjn, zin and r = [zjn; zin],
S = r@r.T / T, pos_i = (zjn_i . zin_i)/T, the kept logits for row i are
the same-view off-diagonal entries plus pos_i.  Since all cosine logits
are <= 1/T = 10 (diagonal exactly 10), use the fixed shift 10:

  lse_i  = 10 + ln( rowsum_i - diag_i + epos_i )
  loss   = mean(lse_i - pos_i)

where rowsum_i = sum_j exp(S_same[i,j] - 10) over the FULL same-view Gram
row (diagonal included), diag_i = exp(10*|qn_i|^2 - 10) computed on host
from the exact bf16 operands, epos_i = exp(pos_i - 10).

Device work per core (SPMD, cores 0-3 view zj, cores 4-7 view zi; each
owns a 1024-row slab): G = qnT.T @ anT (bf16 matmul, fp32 PSUM), then
ACT exp((G)*10 - 10) with accum_out row sums.  Everything O(N*D) or
smaller (normalize, pos, final log/mean) runs on host.
"""

import numpy as np
import ml_dtypes

N = 4096
D = 256
TEMP = 0.1
NCORES = 8
RPC = 2 * N // NCORES          # 1024 rows per core
IT = RPC // 128                # 8 i-tiles of 128 rows
HALF = 2048                    # j-chunk per PSUM buffer / ACT op
NH = N // HALF                 # 2 halves of the 4096-wide Gram row

_CACHE = {}


def _build_program():
    if "nc" in _CACHE:
        return _CACHE["nc"]

    import concourse.bass as bass
    import concourse.tile as tile
    from concourse import bacc, mybir

    BF16 = mybir.dt.bfloat16
    F32 = mybir.dt.float32

    nc = bacc.Bacc(
        "TRN2", target_bir_lowering=False, debug=False, num_devices=NCORES
    )

    # [k][h][p][col]: anT[k*128+p, h*2048+col], contiguous per (k,h) tile
    anT_d = nc.dram_tensor("anT", [2, NH, 128, HALF], BF16, kind="ExternalInput")
    qnT_d = nc.dram_tensor("qnT", [2, 128, RPC], BF16, kind="ExternalInput")
    rs_d = nc.dram_tensor("rs", [128, IT], F32, kind="ExternalOutput")

    with tile.TileContext(nc) as tc:
        with (
            tc.tile_pool(name="weights", bufs=1) as wpool,
            tc.tile_pool(name="scratch", bufs=2) as spool,
            tc.tile_pool(name="psum", bufs=2, space="PSUM") as ppool,
        ):
            qnT = [wpool.tile([128, RPC], BF16, name=f"qnT{k}") for k in range(2)]
            anT = [
                [wpool.tile([128, HALF], BF16, name=f"anT{k}_{h}") for h in range(NH)]
                for k in range(2)
            ]
            for k in range(2):
                nc.default_dma_engine.dma_start(out=qnT[k][:], in_=qnT_d[k])
            for h in range(NH):
                for k in range(2):
                    nc.default_dma_engine.dma_start(out=anT[k][h][:], in_=anT_d[k, h])

            acc = wpool.tile([128, IT, NH], F32)
            rs = wpool.tile([128, IT], F32)
            bias = wpool.tile([128, 1], F32)
            nc.vector.memset(bias[:], -1.0 / TEMP)

            for h in range(NH):
                for t in range(IT):
                    ps = ppool.tile([128, HALF], F32)
                    for k in range(2):
                        for c in range(HALF // 512):
                            nc.tensor.matmul(
                                ps[:, c * 512:(c + 1) * 512],
                                qnT[k][:, t * 128:(t + 1) * 128],
                                anT[k][h][:, c * 512:(c + 1) * 512],
                                start=(k == 0),
                                stop=(k == 1),
                            )
                    sc = spool.tile([128, HALF], BF16)
                    nc.scalar.activation(
                        sc[:],
                        ps[:],
                        mybir.ActivationFunctionType.Exp,
                        bias=bias[:],
                        scale=1.0 / TEMP,
                        accum_out=acc[:, t, h:h + 1],
                    )

            nc.vector.tensor_reduce(
                rs[:], acc[:], axis=mybir.AxisListType.X, op=mybir.AluOpType.add
            )
            nc.default_dma_engine.dma_start(out=rs_d[:], in_=rs[:])

    nc.compile()
    _CACHE["nc"] = nc
    return nc


def _prep_inputs(z_i, z_j):
    zin = z_i / np.sqrt(np.sum(z_i * z_i, axis=1, keepdims=True))
    zjn = z_j / np.sqrt(np.sum(z_j * z_j, axis=1, keepdims=True))
    posn = np.sum(zin * zjn, axis=1, dtype=np.float64) / TEMP      # [4096]

    bf = [zjn.astype(ml_dtypes.bfloat16), zin.astype(ml_dtypes.bfloat16)]
    dsq = [
        np.sum(b.astype(np.float32) ** 2, axis=1, dtype=np.float64) for b in bf
    ]  # exact |qn_i|^2 of the bf16 rows, per view

    in_maps = []
    for c in range(NCORES):
        v = c // (NCORES // 2)
        s = c % (NCORES // 2)
        b = bf[v]
        qnT = np.ascontiguousarray(b[s * RPC:(s + 1) * RPC].T).reshape(2, 128, RPC)
        anT = np.ascontiguousarray(
            b.T.reshape(2, 128, NH, HALF).transpose(0, 2, 1, 3)
        )
        in_maps.append({"anT": anT, "qnT": qnT})
    return in_maps, posn, dsq


def kernel(z_i, z_j):
    z_i = np.asarray(z_i, dtype=np.float32)
    z_j = np.asarray(z_j, dtype=np.float32)

    from concourse.bass_utils import run_bass_kernel_spmd

    nc = _build_program()
    in_maps, posn, dsq = _prep_inputs(z_i, z_j)

    res = run_bass_kernel_spmd(nc, in_maps, list(range(NCORES)))
    _CACHE["last_results"] = res

    rowsum = np.empty(2 * N, dtype=np.float64)
    for c in range(NCORES):
        slab = res.results[c]["rs"].astype(np.float64).T.reshape(-1)  # [1024]
        rowsum[c * RPC:(c + 1) * RPC] = slab

    posn_g = np.concatenate([posn, posn])
    diag_g = np.exp(TEMP ** -1 * np.concatenate(dsq) - 1.0 / TEMP)
    epos_g = np.exp(posn_g - 1.0 / TEMP)

    lse = 1.0 / TEMP + np.log(rowsum - diag_g + epos_g)
    loss = np.mean(lse - posn_g)
    return np.array(loss, dtype=np.float32)
